# revision 6
# baseline (speedup 1.0000x reference)
"""Causal single-head attention on 8 Trainium2 NeuronCores, x-pair-exchange.

Problem: x [4, 2048, 1024], w_q/w_k/w_v [1024, 1024] (nn.Linear convention,
y = x @ W.T). Computes q,k,v projections, causal softmax(q k^T / sqrt(D)) @ v.

Math: scores S[i,t] = q_i . k_t = x_i^T (W_q^T W_k) x_t. The host
precomputes M = W_q^T W_k once (cheap: one 1024^3 sgemm), the device
computes z = M^T x_q per query and contracts S^T[t,i] = x_t . z_i - the
K projection disappears entirely (its FLOPs fold into M on the host), and
the pair-exchange ships RAW x^T instead of K^T, so the first AllGather
triggers ~8us into the kernel instead of after a 30us projection.

Sharding: 2 cores per batch element. Core parity p owns token half
H_p = [p*1024,(p+1)*1024): it forwards x_own^T into two 1MB AllGathers
(tokens [0:512] then [512:1024], replica groups [[0,1],[2,3],[4,5],[6,7]])
and computes V for its half, exchanged via two more 1MB AllGathers fenced
behind the x AGs by data-dependency rows (2 concurrent pair-AGs are safe,
3+ corrupt the odd member - measured). Queries: parity-interleaved
128-tiles (slot k has a kv window of 256k tokens), host-gathered.

Attention computes S^T (token-chunk-major) so the softmax exp output IS
the P^T layout the AV matmul needs - no transposes. One token chunk c
serves every slot k >= floor(c/2)+1, whose query columns are contiguous in
z^T, so each chunk is 1-2 wide matmuls per e-chunk instead of one per
slot. exp reads straight from PSUM (no max subtraction - scores/sqrt(D)
are O(1)); per-slot row sums come from accumulating ones-matmuls; the
causal mask (chunk c is the boundary of exactly its first slot column
block) is one extra accumulation matmul of identity @ maskT. AV runs as
two passes (V-a chunks for all slots, then V-b) so the last AG can arrive
late without stalling the PE queue; the O scale/merge runs on the DVE.

All matmul operands are bf16; softmax statistics and PSUM stay f32.
Every DMA is a 2D [128, W] transfer with a contiguous DRAM slab -
strided 3D DMAs generate descriptors on the triggering engine at
~1us/KB-of-fragmentation and are a trap (measured). DMA triggers cost
~600ns of engine queue time each, so loads are split across both HWDGE
engines and elementwise work is kept off the Scalar queue.
"""
import numpy as np
import ml_dtypes
from contextlib import ExitStack

import concourse.bass as bass
import concourse.tile as tile
import concourse.mybir as mybir
from concourse.bass_utils import run_bass_kernel_spmd
from concourse.masks import make_identity

F32 = mybir.dt.float32
BF16 = mybir.dt.bfloat16
AF = mybir.ActivationFunctionType
AX = mybir.AxisListType

B, S, E, D = 4, 2048, 1024, 1024
NCORES = 8
NSLOT = 8              # slots k=1..8, kv window = 256*k tokens
NQ = NSLOT * 128       # queries per core
HT = S // 2            # tokens owned per core (own half)
HH = HT // 2           # token quarter (AG granularity)
EC = E // 128          # e-chunks
NCH = S // 128         # token chunks
SCALE = 1.0 / 32.0     # 1/sqrt(D)
MASKVAL = -30000.0
GROUPS = [[0, 1], [2, 3], [4, 5], [6, 7]]

_prog = None


def _kmin(c):
    """First slot whose kv window includes token chunk c."""
    return c // 2 + 1


def _split_multi_waits(nc, max_waits=1):
    """The walrus build in this container has one sync-wait slot per
    instruction; hoist extra waits onto preceding same-engine NoOps."""
    n = 0
    for f in nc.m.functions:
        for b in f.blocks:
            insts = b.instructions
            out = []
            changed = False
            for ins in insts:
                si = ins.sync_info
                if si is not None and len(si.on_wait) > max_waits:
                    waits = list(si.on_wait)
                    for w in waits[:-max_waits]:
                        nop = mybir.InstNoOp(name=f"I-waitsplit-{n}")
                        n += 1
                        nop.engine = ins.engine
                        nop.sync_info = mybir.SyncInfo(on_wait=[w], on_update=[])
                        out.append(nop)
                    ins.sync_info = mybir.SyncInfo(
                        on_wait=waits[-max_waits:], on_update=list(si.on_update))
                    changed = True
                out.append(ins)
            if changed:
                b.instructions = out
    return nc


def _build(split=True):
    nc = bass.Bass(trn_type="TRN2", target_bir_lowering=False, debug=False)
    xoT = nc.dram_tensor("xoT", [E, HT], BF16, kind="ExternalInput").ap()
    xqT = nc.dram_tensor("xqT", [E, NQ], BF16, kind="ExternalInput").ap()
    mT = nc.dram_tensor("m", [E, E], BF16, kind="ExternalInput").ap()
    wvT = nc.dram_tensor("wvT", [E, D], BF16, kind="ExternalInput").ap()
    # maskT: transposed causal boundary mask [2*128 window rows, 128 queries]
    maskin = nc.dram_tensor("maskT", [256, 128], BF16, kind="ExternalInput").ap()
    onesin = nc.dram_tensor("ones", [128, 1], BF16, kind="ExternalInput").ap()
    out = nc.dram_tensor("out", [NQ, D], F32, kind="ExternalOutput").ap()

    bncX, gathX = [], []
    for g in range(2):
        bncX.append(nc.dram_tensor(f"bncX{g}", [EC, 128, HH], BF16).ap())
        gathX.append(nc.dram_tensor(f"gathX{g}", [2, EC, 128, HH], BF16).ap())
    bncV, gathV = [], []
    for v in range(2):
        bncV.append(nc.dram_tensor(f"bncV{v}", [HH + 1, D], BF16).ap())
        gathV.append(nc.dram_tensor(f"gathV{v}", [2, HH + 1, D], BF16).ap())

    with tile.TileContext(nc) as tc, ExitStack() as ctx:
        # x^T halves straight into the AG bounce buffers (DRAM->DRAM, no
        # compute dependency) - the x AGs trigger ~8us into the kernel
        for g in range(2):
            for e in range(EC):
                nc.scalar.dma_start(bncX[g][e],
                                    xoT[e * 128:(e + 1) * 128,
                                        g * HH:(g + 1) * HH])
            nc.gpsimd.collective_compute(
                "AllGather", mybir.AluOpType.bypass, replica_groups=GROUPS,
                ins=[bncX[g].opt()], outs=[gathX[g].opt()])

        const = ctx.enter_context(tc.tile_pool(name="const", bufs=1))
        ident = const.tile([128, 128], BF16)
        make_identity(nc, ident[:])
        maskT = const.tile([128, 256], BF16)   # [:, 0:128]=rows 0:128, etc
        nc.scalar.dma_start(maskT[:, 0:128], maskin[0:128, :])
        nc.scalar.dma_start(maskT[:, 128:256], maskin[128:256, :])
        ones = const.tile([128, 1], BF16)
        nc.scalar.dma_start(ones[:], onesin[:])

        # z^T stays resident until the end of attention. col = e*NQ + q
        qtp = ctx.enter_context(tc.tile_pool(name="qtp", bufs=1))
        zts = qtp.tile([128, EC * NQ], BF16, name="zts")

        # ---- Phase 1: V_own -> AGs (by token quarter), then z = M^T x_q ----
        with tc.tile_pool(name="wp", bufs=1) as wp, \
             tc.tile_pool(name="xp", bufs=1) as xp, \
             tc.tile_pool(name="st", bufs=1) as stp, \
             tc.tile_pool(name="ps1", bufs=4, space="PSUM") as pp:
            # fused weight/activation tiles: col = e*width + c
            wv = wp.tile([128, EC * D], BF16, name="wv")
            m = wp.tile([128, EC * E], BF16, name="m")
            xo = xp.tile([128, EC * HT], BF16, name="xo")
            xq = xp.tile([128, EC * NQ], BF16, name="xq")

            # startup: per-e 2D loads, xo/wv interleaved first (V runs first)
            for e in range(EC):
                nc.sync.dma_start(xo[:, e * HT:(e + 1) * HT],
                                  xoT[e * 128:(e + 1) * 128, :])
                nc.sync.dma_start(wv[:, e * D:(e + 1) * D],
                                  wvT[e * 128:(e + 1) * 128, :])
            for e in range(EC):
                nc.sync.dma_start(m[:, e * E:(e + 1) * E],
                                  mT[e * 128:(e + 1) * 128, :])
            for e in range(EC):
                nc.sync.dma_start(xq[:, e * NQ:(e + 1) * NQ],
                                  xqT[e * 128:(e + 1) * 128, :])

            # V_own: stationary x chunks, moving wv; token quarter v first.
            # vown col = t*D + c  (t = own-half token chunk 0..7)
            vown = stp.tile([128, (HT // 128) * D], BF16, name="vown")
            for v in range(2):
                for tl in range(HH // 128):
                    t = v * (HH // 128) + tl
                    for h in range(2):
                        ps = pp.tile([128, 512], F32, name=f"pv{t}_{h}", tag="pp")
                        for e in range(EC):
                            nc.tensor.matmul(
                                ps[:],
                                xo[:, e * HT + t * 128:e * HT + (t + 1) * 128],
                                wv[:, e * D + h * 512:e * D + (h + 1) * 512],
                                start=(e == 0), stop=(e == EC - 1))
                        nc.vector.tensor_copy(
                            vown[:, t * D + h * 512:t * D + (h + 1) * 512],
                            ps[:])
                    nc.scalar.dma_start(bncV[v][tl * 128:(tl + 1) * 128, :],
                                        vown[:, t * D:(t + 1) * D])
                # fence: the V AG may only trigger once the same-index x AG
                # has fully delivered (reads replica-1 bytes of its output)
                nc.scalar.dma_start(bncV[v][HH:HH + 1, 0:16],
                                    gathX[v][1, 0, 0:1, 0:16])
                nc.gpsimd.collective_compute(
                    "AllGather", mybir.AluOpType.bypass, replica_groups=GROUPS,
                    ins=[bncV[v].opt()], outs=[gathV[v].opt()])

            # z^T = M^T x_q: stationary M chunks, moving xq. col = e*NQ + q.
            for d in range(EC):
                for g in range(2):
                    ps = pp.tile([128, 512], F32, name=f"pq{d}_{g}", tag="pp")
                    for e in range(EC):
                        nc.tensor.matmul(
                            ps[:],
                            m[:, e * E + d * 128:e * E + (d + 1) * 128],
                            xq[:, e * NQ + g * 512:e * NQ + (g + 1) * 512],
                            start=(e == 0), stop=(e == EC - 1))
                    nc.vector.tensor_copy(
                        zts[:, d * NQ + g * 512:d * NQ + (g + 1) * 512], ps[:])

        # ---- Phase 2: load gathered x^T / V into SBUF (2D DMAs) ----
        # xts col = e*S + t (global token order); vts col = t*D + c
        kvp = ctx.enter_context(tc.tile_pool(name="kvp", bufs=1))
        xts = kvp.tile([128, EC * S], BF16, name="xts")
        vts = kvp.tile([128, NCH * D], BF16, name="vts")
        for g in range(2):
            for r in range(2):
                base = r * HT + g * HH
                for e in range(EC):
                    eng = nc.sync if e % 2 == 0 else nc.scalar
                    eng.dma_start(xts[:, e * S + base:e * S + base + HH],
                                  gathX[g][r, e])
        for v in range(2):
            for r in range(2):
                for tl in range(HH // 128):
                    t = r * 8 + v * 4 + tl   # global chunk
                    eng = nc.sync if tl % 2 == 0 else nc.scalar
                    eng.dma_start(vts[:, t * D:(t + 1) * D],
                                  gathV[v][r, tl * 128:(tl + 1) * 128, :])

        # ---- Phase 3: chunk-major S^T scores + softmax (P^T straight) ----
        att = ctx.enter_context(tc.tile_pool(name="att", bufs=1))
        stats = ctx.enter_context(tc.tile_pool(name="stats", bufs=1))
        linv = stats.tile([128, NSLOT], F32, name="linv")
        # per-chunk P^T tiles: cols = slots kmin(c)..8, 128 each
        pT = {c: att.tile([128, 128 * (NSLOT + 1 - _kmin(c))], BF16,
                          name=f"pT{c}") for c in range(NCH)}
        osb = {k: att.tile([128, D], F32, name=f"osb{k}")
               for k in range(3, NSLOT + 1)}
        av_a = {k: [c for c in range(2 * k) if c % 8 < 4]
                for k in range(1, NSLOT + 1)}
        av_b = {k: [c for c in range(2 * k) if c % 8 >= 4]
                for k in range(1, NSLOT + 1)}

        with tc.tile_pool(name="ps3", bufs=1, space="PSUM") as pp3:
            ls = pp3.tile([128, 2], F32, name="ls", tag="lsp", bufs=1)

            def emit_lsum(k):
                for ci, c in enumerate(range(2 * k)):
                    j = k - _kmin(c)
                    nc.tensor.matmul(ls[:, 0:1],
                                     pT[c][:, j * 128:(j + 1) * 128],
                                     ones[:], start=(ci == 0),
                                     stop=(ci == 2 * k - 1))
                nc.vector.reciprocal(linv[:, k - 1:k], ls[:, 0:1])

            for c in range(NCH):
                km = _kmin(c)
                w = 128 * (NSLOT + 1 - km)
                npc = (w + 511) // 512
                sT = [pp3.tile([128, 512], F32, name=f"sT{c}_{i}", tag="sps",
                               bufs=3) for i in range(npc)]
                for i in range(npc):
                    pw = min(512, w - i * 512)
                    qoff = (km - 1) * 128 + i * 512
                    msk = (i == 0)
                    for e in range(EC):
                        nc.tensor.matmul(
                            sT[i][:, :pw],
                            xts[:, e * S + c * 128:e * S + (c + 1) * 128],
                            zts[:, e * NQ + qoff:e * NQ + qoff + pw],
                            start=(e == 0), stop=(e == EC - 1 and not msk))
                    if msk:
                        # chunk c is the causal boundary of slot kmin(c),
                        # which owns this chunk's first 128 query columns
                        mo = 0 if c % 2 == 0 else 128
                        nc.tensor.matmul(sT[i][:, 0:128], ident[:],
                                         maskT[:, mo:mo + 128],
                                         start=False, stop=True,
                                         skip_group_check=True)
                    nc.scalar.activation(pT[c][:, i * 512:i * 512 + pw],
                                         sT[i][:, :pw], AF.Exp, scale=SCALE)
                # slot k's last chunk is 2k-1; emit its row-sum matmuls one
                # chunk later so the PE never waits on the exp it just fed
                if c >= 2 and c % 2 == 0:
                    emit_lsum(c // 2)
            emit_lsum(NSLOT)

            # ---- Phase 4a: AV over the V-a chunks for every slot ----
            for k in range(1, NSLOT + 1):
                ca = av_a[k]
                o_ps = [pp3.tile([128, 512], F32, name=f"oa{k}_{h}", tag="ops",
                                 bufs=4) for h in range(2)]
                for ci, c in enumerate(ca):
                    j = k - _kmin(c)
                    for h in range(2):
                        nc.tensor.matmul(o_ps[h][:],
                                         pT[c][:, j * 128:(j + 1) * 128],
                                         vts[:, c * D + h * 512:c * D + (h + 1) * 512],
                                         start=(ci == 0), stop=(ci == len(ca) - 1))
                if not av_b[k]:
                    # k=1,2: fully V-a resident; normalize and write out
                    o_fin = att.tile([128, D], F32, name=f"ofa{k}", tag="ofin",
                                     bufs=2)
                    for h in range(2):
                        nc.scalar.activation(o_fin[:, h * 512:(h + 1) * 512],
                                             o_ps[h][:], AF.Copy,
                                             scale=linv[:, k - 1:k])
                    nc.sync.dma_start(out[(k - 1) * 128:k * 128, :], o_fin[:])
                else:
                    # stash normalized partial on the DVE; V-b added in 4b
                    for h in range(2):
                        nc.vector.tensor_scalar_mul(
                            osb[k][:, h * 512:(h + 1) * 512], o_ps[h][:],
                            linv[:, k - 1:k])

            # ---- Phase 4b: AV over the V-b chunks, scale+add on DVE ----
            for k in range(3, NSLOT + 1):
                cb = av_b[k]
                o_ps = [pp3.tile([128, 512], F32, name=f"ob{k}_{h}", tag="ops",
                                 bufs=4) for h in range(2)]
                for ci, c in enumerate(cb):
                    j = k - _kmin(c)
                    for h in range(2):
                        nc.tensor.matmul(o_ps[h][:],
                                         pT[c][:, j * 128:(j + 1) * 128],
                                         vts[:, c * D + h * 512:c * D + (h + 1) * 512],
                                         start=(ci == 0), stop=(ci == len(cb) - 1))
                o_fin = att.tile([128, D], F32, name=f"ofb{k}", tag="ofin",
                                 bufs=2)
                o_sc = att.tile([128, D], F32, name=f"osc{k}", tag="osc", bufs=2)
                for h in range(2):
                    hs = slice(h * 512, (h + 1) * 512)
                    nc.vector.tensor_scalar_mul(o_sc[:, hs], o_ps[h][:],
                                                linv[:, k - 1:k])
                    nc.vector.tensor_add(o_fin[:, hs], o_sc[:, hs], osb[k][:, hs])
                nc.sync.dma_start(out[(k - 1) * 128:k * 128, :], o_fin[:])
    if split:
        _split_multi_waits(nc)
    return nc


def _masks():
    """Transposed boundary masks [256 window rows, 128 query cols], bf16."""
    j = np.arange(256)[:, None]
    i = np.arange(128)[None, :]
    bf = ml_dtypes.bfloat16
    maskT0 = np.where(j <= i, 0.0, MASKVAL).astype(bf)          # parity 0
    maskT1 = np.where(j <= 128 + i, 0.0, MASKVAL).astype(bf)    # parity 1
    return maskT0, maskT1


def _in_maps(x, w_q, w_k, w_v):
    bf = ml_dtypes.bfloat16
    x = np.asarray(x, np.float32)
    # fold the Q and K projections into one host-side matrix:
    # S[i,t] = x_i^T (W_q^T W_k) x_t
    m = np.ascontiguousarray(
        np.asarray(w_q, np.float32).T @ np.asarray(w_k, np.float32)).astype(bf)
    wvT = np.ascontiguousarray(np.asarray(w_v, np.float32).T).astype(bf)
    maskT0, maskT1 = _masks()
    ones = np.ones((128, 1), dtype=bf)

    in_maps = []
    for c in range(NCORES):
        b, p = divmod(c, 2)
        xb = x[b]                                    # [S, E]
        xoT = np.ascontiguousarray(xb[p * HT:(p + 1) * HT, :].T).astype(bf)
        qrows = np.concatenate(
            [xb[128 * (2 * (k - 1) + p):128 * (2 * (k - 1) + p) + 128, :]
             for k in range(1, NSLOT + 1)], axis=0)  # [NQ, E]
        xqT = np.ascontiguousarray(qrows.T).astype(bf)
        in_maps.append({
            "xoT": xoT, "xqT": xqT,
            "m": m, "wvT": wvT,
            "maskT": maskT0 if p == 0 else maskT1,
            "ones": ones,
        })
    return in_maps


def _scatter(per_core_out):
    out = np.empty((B, S, D), dtype=np.float32)
    for c in range(NCORES):
        b, p = divmod(c, 2)
        oc = per_core_out[c]                         # [NQ, D]
        for k in range(1, NSLOT + 1):
            g = 2 * (k - 1) + p
            out[b, 128 * g:128 * (g + 1), :] = oc[128 * (k - 1):128 * k, :]
    return out


def kernel(x, w_q, w_k, w_v):
    global _prog
    if _prog is None:
        _prog = _build()
    in_maps = _in_maps(x, w_q, w_k, w_v)
    res = run_bass_kernel_spmd(_prog, in_maps, list(range(NCORES)))
    return _scatter([res.results[c]["out"] for c in range(NCORES)])


# revision 7
# speedup vs baseline: 1.2711x; 1.2711x over previous
"""Causal single-head attention on 8 Trainium2 NeuronCores, x-pair-exchange.

Problem: x [4, 2048, 1024], w_q/w_k/w_v [1024, 1024] (nn.Linear convention,
y = x @ W.T). Computes q,k,v projections, causal softmax(q k^T / sqrt(D)) @ v.

Math: scores S[i,t] = q_i . k_t = x_i^T (W_q^T W_k) x_t. The host
precomputes M = W_q^T W_k once (one 1024^3 sgemm - pure weight
preprocessing), the device computes z = M^T x_q per query and contracts
S^T[t,i] = x_t . z_i: the K projection disappears (folded into M) and the
pair-exchange ships RAW x^T instead of K^T, so the first AllGather
triggers ~5us into the kernel with no compute dependency.

Sharding: 2 cores per batch element. Core parity p owns token half
H_p = [p*1024,(p+1)*1024): it forwards x_own^T into two 1MB AllGathers
(tokens [0:512] then [512:1024], replica groups [[0,1],[2,3],[4,5],[6,7]])
and computes V for its half, exchanged via two more 1MB AllGathers fenced
behind the x AGs by data-dependency bytes in their bounce buffers (2
concurrent pair-AGs are safe, 3+ corrupt the odd member - measured).
Queries: parity-interleaved 128-tiles (slot k has a kv window of 256k
tokens), host-gathered.

Attention computes S^T (token-chunk-major) so the softmax exp output IS
the P^T layout the AV matmul needs - no transposes. One token chunk c
serves every slot k >= floor(c/2)+1, whose query columns are contiguous in
z^T, so each chunk is 1-2 wide matmuls per e-chunk instead of one per
slot. exp reads straight from PSUM (no max subtraction - scores/sqrt(D)
are O(1)); per-slot row sums are accumulating ones-matmuls (~28ns each);
the causal mask (chunk c is the boundary of exactly its first slot column
block) is one extra accumulation matmul of identity @ maskT. AV runs as
two passes (V-a chunks for all slots, then V-b) so the last AG can arrive
late without stalling the PE queue; O scale/merge runs on the DVE.

DMA discipline (measured the hard way): a DMA trigger costs ~650ns of
HWDGE engine queue time PLUS ~2.5-5ns per contiguous run of descriptor
generation, so EVERY bulk tensor here - inputs, bounce, gather, loads -
is laid out partition-major [128, W] so each transfer is one DMA with 128
large runs. The host pre-tiles all inputs into SBUF layout. Total DMA
count ~30 (a naive per-piece version needs ~120 and loses ~40us of
engine-queue time to triggers alone, starving the PE's exp/psum chains).
"""
import numpy as np
import ml_dtypes
from contextlib import ExitStack

import concourse.bass as bass
import concourse.tile as tile
import concourse.mybir as mybir
from concourse.bass_utils import run_bass_kernel_spmd
from concourse.masks import make_identity

F32 = mybir.dt.float32
BF16 = mybir.dt.bfloat16
AF = mybir.ActivationFunctionType
AX = mybir.AxisListType

B, S, E, D = 4, 2048, 1024, 1024
NCORES = 8
NSLOT = 8              # slots k=1..8, kv window = 256*k tokens
NQ = NSLOT * 128       # queries per core
HT = S // 2            # tokens owned per core (own half)
HH = HT // 2           # token quarter (AG granularity)
EC = E // 128          # e-chunks
NCH = S // 128         # token chunks
QW = EC * HH           # columns per x^T token-quarter tile (4096)
SCALE = 1.0 / 32.0     # 1/sqrt(D)
MASKVAL = -30000.0
GROUPS = [[0, 1], [2, 3], [4, 5], [6, 7]]

_prog = None


def _kmin(c):
    """First slot whose kv window includes token chunk c."""
    return c // 2 + 1


def _split_multi_waits(nc, max_waits=1):
    """The walrus build in this container has one sync-wait slot per
    instruction; hoist extra waits onto preceding same-engine NoOps."""
    n = 0
    for f in nc.m.functions:
        for b in f.blocks:
            insts = b.instructions
            out = []
            changed = False
            for ins in insts:
                si = ins.sync_info
                if si is not None and len(si.on_wait) > max_waits:
                    waits = list(si.on_wait)
                    for w in waits[:-max_waits]:
                        nop = mybir.InstNoOp(name=f"I-waitsplit-{n}")
                        n += 1
                        nop.engine = ins.engine
                        nop.sync_info = mybir.SyncInfo(on_wait=[w], on_update=[])
                        out.append(nop)
                    ins.sync_info = mybir.SyncInfo(
                        on_wait=waits[-max_waits:], on_update=list(si.on_update))
                    changed = True
                out.append(ins)
            if changed:
                b.instructions = out
    return nc


def _build(split=True):
    nc = bass.Bass(trn_type="TRN2", target_bir_lowering=False, debug=False)
    # all bulk inputs are host-pre-tiled to partition-major SBUF layout
    xo_in = [nc.dram_tensor(f"xo{g}", [128, QW], BF16,
                            kind="ExternalInput").ap() for g in range(2)]
    xq_in = nc.dram_tensor("xq", [128, EC * NQ], BF16, kind="ExternalInput").ap()
    m_in = nc.dram_tensor("m", [128, EC * E], BF16, kind="ExternalInput").ap()
    wv_in = nc.dram_tensor("wv", [128, EC * D], BF16, kind="ExternalInput").ap()
    maskin = nc.dram_tensor("maskT", [256, 128], BF16, kind="ExternalInput").ap()
    onesin = nc.dram_tensor("ones", [128, 1], BF16, kind="ExternalInput").ap()
    out = nc.dram_tensor("out", [NQ, D], F32, kind="ExternalOutput").ap()

    bncX, gathX = [], []
    for g in range(2):
        bncX.append(nc.dram_tensor(f"bncX{g}", [128, QW], BF16).ap())
        gathX.append(nc.dram_tensor(f"gathX{g}", [2, 128, QW], BF16).ap())
    bncV, gathV = [], []
    for v in range(2):
        bncV.append(nc.dram_tensor(f"bncV{v}", [128, 4 * D + 16], BF16).ap())
        gathV.append(nc.dram_tensor(f"gathV{v}", [2, 128, 4 * D + 16], BF16).ap())

    with tile.TileContext(nc) as tc, ExitStack() as ctx:
        # x^T halves straight into the AG bounce buffers (DRAM->DRAM
        # contiguous, no compute dependency): x AGs trigger ~5us in
        for g in range(2):
            nc.scalar.dma_start(bncX[g][:], xo_in[g][:])
            nc.gpsimd.collective_compute(
                "AllGather", mybir.AluOpType.bypass, replica_groups=GROUPS,
                ins=[bncX[g].opt()], outs=[gathX[g].opt()])

        const = ctx.enter_context(tc.tile_pool(name="const", bufs=1))
        ident = const.tile([128, 128], BF16)
        make_identity(nc, ident[:])
        maskT = const.tile([128, 256], BF16)   # [:, 0:128]=rows 0:128, etc
        nc.scalar.dma_start(maskT[:, 0:128], maskin[0:128, :])
        nc.scalar.dma_start(maskT[:, 128:256], maskin[128:256, :])
        ones = const.tile([128, 1], BF16)
        nc.scalar.dma_start(ones[:], onesin[:])

        # z^T stays resident until the end of attention. col = e*NQ + q
        qtp = ctx.enter_context(tc.tile_pool(name="qtp", bufs=1))
        zts = qtp.tile([128, EC * NQ], BF16, name="zts")

        # ---- Phase 1: V_own -> AGs (by token quarter), then z = M^T x_q ----
        with tc.tile_pool(name="wp", bufs=1) as wp, \
             tc.tile_pool(name="xp", bufs=1) as xp, \
             tc.tile_pool(name="st", bufs=1) as stp, \
             tc.tile_pool(name="ps1", bufs=4, space="PSUM") as pp:
            # xo col = g*QW + e*HH + t ; wv/m cols = e*1024 + c
            wv = wp.tile([128, EC * D], BF16, name="wv")
            m = wp.tile([128, EC * E], BF16, name="m")
            xo = xp.tile([128, 2 * QW], BF16, name="xo")
            xq = xp.tile([128, EC * NQ], BF16, name="xq")

            # few, large loads; e0-3 pieces of xo-g0/wv first so the V
            # matmuls can start while the rest stream in
            half = EC * D // 2
            nc.sync.dma_start(xo[:, 0:QW // 2], xo_in[0][:, 0:QW // 2])
            nc.sync.dma_start(wv[:, 0:half], wv_in[:, 0:half])
            nc.sync.dma_start(xo[:, QW // 2:QW], xo_in[0][:, QW // 2:QW])
            nc.sync.dma_start(wv[:, half:], wv_in[:, half:])
            nc.sync.dma_start(xo[:, QW:2 * QW], xo_in[1][:])
            nc.sync.dma_start(m[:, 0:half], m_in[:, 0:half])
            nc.sync.dma_start(m[:, half:], m_in[:, half:])
            nc.sync.dma_start(xq[:, 0:half], xq_in[:, 0:half])
            nc.sync.dma_start(xq[:, half:], xq_in[:, half:])

            # V_own: stationary x chunks, moving wv; token quarter v first.
            # vown col = t*D + c  (t = own-half token chunk 0..7)
            vown = stp.tile([128, (HT // 128) * D], BF16, name="vown")
            for v in range(2):
                for tl in range(HH // 128):
                    t = v * (HH // 128) + tl
                    xcol = v * QW + tl * 128          # x^T col of this chunk
                    for h in range(2):
                        ps = pp.tile([128, 512], F32, name=f"pv{t}_{h}", tag="pp")
                        for e in range(EC):
                            nc.tensor.matmul(
                                ps[:],
                                xo[:, xcol + e * HH:xcol + e * HH + 128],
                                wv[:, e * D + h * 512:e * D + (h + 1) * 512],
                                start=(e == 0), stop=(e == EC - 1))
                        nc.vector.tensor_copy(
                            vown[:, t * D + h * 512:t * D + (h + 1) * 512],
                            ps[:])
                # one contiguous bounce DMA for the whole quarter
                nc.scalar.dma_start(bncV[v][:, 0:4 * D],
                                    vown[:, v * 4 * D:(v + 1) * 4 * D])
                # fence: the V AG may only trigger once the same-index x AG
                # has fully delivered (reads replica-1 bytes of its output)
                nc.scalar.dma_start(bncV[v][0:1, 4 * D:4 * D + 16],
                                    gathX[v][1, 0:1, 0:16])
                nc.gpsimd.collective_compute(
                    "AllGather", mybir.AluOpType.bypass, replica_groups=GROUPS,
                    ins=[bncV[v].opt()], outs=[gathV[v].opt()])

            # z^T = M^T x_q: stationary M chunks, moving xq. col = e*NQ + q.
            for d in range(EC):
                for g in range(2):
                    ps = pp.tile([128, 512], F32, name=f"pq{d}_{g}", tag="pp")
                    for e in range(EC):
                        nc.tensor.matmul(
                            ps[:],
                            m[:, e * E + d * 128:e * E + (d + 1) * 128],
                            xq[:, e * NQ + g * 512:e * NQ + (g + 1) * 512],
                            start=(e == 0), stop=(e == EC - 1))
                    nc.vector.tensor_copy(
                        zts[:, d * NQ + g * 512:d * NQ + (g + 1) * 512], ps[:])

        # ---- Phase 2: load gathered x^T / V into SBUF (one DMA each) ----
        # xts: token-quarter-major: col = q4*QW + e*HH + tq
        #   (global quarter q4 = r*2 + g; chunk c -> q4=c//4, tq=(c%4)*128)
        # vts col = t*D + c  (global chunk t)
        kvp = ctx.enter_context(tc.tile_pool(name="kvp", bufs=1))
        xts = kvp.tile([128, 4 * QW], BF16, name="xts")
        vts = kvp.tile([128, NCH * D], BF16, name="vts")
        for g in range(2):
            for r in range(2):
                q4 = r * 2 + g
                nc.sync.dma_start(xts[:, q4 * QW:(q4 + 1) * QW], gathX[g][r])
        for v in range(2):
            for r in range(2):
                t0 = r * 8 + v * 4
                nc.sync.dma_start(vts[:, t0 * D:(t0 + 4) * D],
                                  gathV[v][r, :, 0:4 * D])

        def xtc(c, e):
            """xts col of (global token chunk c, e-chunk e)."""
            return (c // 4) * QW + e * HH + (c % 4) * 128

        # ---- Phase 3: chunk-major S^T scores + softmax (P^T straight) ----
        att = ctx.enter_context(tc.tile_pool(name="att", bufs=1))
        stats = ctx.enter_context(tc.tile_pool(name="stats", bufs=1))
        linv = stats.tile([128, NSLOT], F32, name="linv")
        # per-chunk P^T tiles: cols = slots kmin(c)..8, 128 each
        pT = {c: att.tile([128, 128 * (NSLOT + 1 - _kmin(c))], BF16,
                          name=f"pT{c}") for c in range(NCH)}
        osb = {k: att.tile([128, D], F32, name=f"osb{k}")
               for k in range(3, NSLOT + 1)}
        av_a = {k: [c for c in range(2 * k) if c % 8 < 4]
                for k in range(1, NSLOT + 1)}
        av_b = {k: [c for c in range(2 * k) if c % 8 >= 4]
                for k in range(1, NSLOT + 1)}

        with tc.tile_pool(name="ps3", bufs=1, space="PSUM") as pp3:
            ls = pp3.tile([128, 2], F32, name="ls", tag="lsp", bufs=1)

            def emit_lsum(k):
                for ci, c in enumerate(range(2 * k)):
                    j = k - _kmin(c)
                    nc.tensor.matmul(ls[:, 0:1],
                                     pT[c][:, j * 128:(j + 1) * 128],
                                     ones[:], start=(ci == 0),
                                     stop=(ci == 2 * k - 1))
                nc.vector.reciprocal(linv[:, k - 1:k], ls[:, 0:1])

            for c in range(NCH):
                km = _kmin(c)
                w = 128 * (NSLOT + 1 - km)
                npc = (w + 511) // 512
                sT = [pp3.tile([128, 512], F32, name=f"sT{c}_{i}", tag="sps",
                               bufs=3) for i in range(npc)]
                for i in range(npc):
                    pw = min(512, w - i * 512)
                    qoff = (km - 1) * 128 + i * 512
                    msk = (i == 0)
                    for e in range(EC):
                        nc.tensor.matmul(
                            sT[i][:, :pw],
                            xts[:, xtc(c, e):xtc(c, e) + 128],
                            zts[:, e * NQ + qoff:e * NQ + qoff + pw],
                            start=(e == 0), stop=(e == EC - 1 and not msk))
                    if msk:
                        # chunk c is the causal boundary of slot kmin(c),
                        # which owns this chunk's first 128 query columns
                        mo = 0 if c % 2 == 0 else 128
                        nc.tensor.matmul(sT[i][:, 0:128], ident[:],
                                         maskT[:, mo:mo + 128],
                                         start=False, stop=True,
                                         skip_group_check=True)
                    nc.scalar.activation(pT[c][:, i * 512:i * 512 + pw],
                                         sT[i][:, :pw], AF.Exp, scale=SCALE)
                # slot k's last chunk is 2k-1; emit its row-sum matmuls one
                # chunk later so the PE never waits on the exp it just fed
                if c >= 2 and c % 2 == 0:
                    emit_lsum(c // 2)
            emit_lsum(NSLOT)

            # ---- Phase 4a: AV over the V-a chunks for every slot ----
            for k in range(1, NSLOT + 1):
                ca = av_a[k]
                o_ps = [pp3.tile([128, 512], F32, name=f"oa{k}_{h}", tag="ops",
                                 bufs=4) for h in range(2)]
                for ci, c in enumerate(ca):
                    j = k - _kmin(c)
                    for h in range(2):
                        nc.tensor.matmul(o_ps[h][:],
                                         pT[c][:, j * 128:(j + 1) * 128],
                                         vts[:, c * D + h * 512:c * D + (h + 1) * 512],
                                         start=(ci == 0), stop=(ci == len(ca) - 1))
                if not av_b[k]:
                    # k=1,2: fully V-a resident; normalize and write out
                    o_fin = att.tile([128, D], F32, name=f"ofa{k}", tag="ofin",
                                     bufs=2)
                    for h in range(2):
                        nc.scalar.activation(o_fin[:, h * 512:(h + 1) * 512],
                                             o_ps[h][:], AF.Copy,
                                             scale=linv[:, k - 1:k])
                    nc.sync.dma_start(out[(k - 1) * 128:k * 128, :], o_fin[:])
                else:
                    # stash normalized partial on the DVE; V-b added in 4b
                    for h in range(2):
                        nc.vector.tensor_scalar_mul(
                            osb[k][:, h * 512:(h + 1) * 512], o_ps[h][:],
                            linv[:, k - 1:k])

            # ---- Phase 4b: AV over the V-b chunks, scale+add on DVE ----
            for k in range(3, NSLOT + 1):
                cb = av_b[k]
                o_ps = [pp3.tile([128, 512], F32, name=f"ob{k}_{h}", tag="ops",
                                 bufs=4) for h in range(2)]
                for ci, c in enumerate(cb):
                    j = k - _kmin(c)
                    for h in range(2):
                        nc.tensor.matmul(o_ps[h][:],
                                         pT[c][:, j * 128:(j + 1) * 128],
                                         vts[:, c * D + h * 512:c * D + (h + 1) * 512],
                                         start=(ci == 0), stop=(ci == len(cb) - 1))
                o_fin = att.tile([128, D], F32, name=f"ofb{k}", tag="ofin",
                                 bufs=2)
                o_sc = att.tile([128, D], F32, name=f"osc{k}", tag="osc", bufs=2)
                for h in range(2):
                    hs = slice(h * 512, (h + 1) * 512)
                    nc.vector.tensor_scalar_mul(o_sc[:, hs], o_ps[h][:],
                                                linv[:, k - 1:k])
                    nc.vector.tensor_add(o_fin[:, hs], o_sc[:, hs], osb[k][:, hs])
                nc.sync.dma_start(out[(k - 1) * 128:k * 128, :], o_fin[:])
    if split:
        _split_multi_waits(nc)
    return nc


def _masks():
    """Transposed boundary masks [256 window rows, 128 query cols], bf16."""
    j = np.arange(256)[:, None]
    i = np.arange(128)[None, :]
    bf = ml_dtypes.bfloat16
    maskT0 = np.where(j <= i, 0.0, MASKVAL).astype(bf)          # parity 0
    maskT1 = np.where(j <= 128 + i, 0.0, MASKVAL).astype(bf)    # parity 1
    return maskT0, maskT1


def _ptile(a):
    """[E, W] -> partition-major [128, EC*W]: out[p, e*W+c] = a[e*128+p, c]."""
    Erows, W = a.shape
    ec = Erows // 128
    return np.ascontiguousarray(
        a.reshape(ec, 128, W).transpose(1, 0, 2).reshape(128, ec * W))


def _in_maps(x, w_q, w_k, w_v):
    bf = ml_dtypes.bfloat16
    x = np.asarray(x, np.float32)
    # fold the Q and K projections into one host-side matrix:
    # S[i,t] = x_i^T (W_q^T W_k) x_t
    m = (np.asarray(w_q, np.float32).T @ np.asarray(w_k, np.float32))
    m_t = _ptile(m.astype(bf))
    wv_t = _ptile(np.ascontiguousarray(np.asarray(w_v, np.float32).T).astype(bf))
    maskT0, maskT1 = _masks()
    ones = np.ones((128, 1), dtype=bf)

    in_maps = []
    for c in range(NCORES):
        b, p = divmod(c, 2)
        xb = x[b]                                    # [S, E]
        xoT = np.ascontiguousarray(xb[p * HT:(p + 1) * HT, :].T).astype(bf)
        xo0 = _ptile(np.ascontiguousarray(xoT[:, 0:HH]))
        xo1 = _ptile(np.ascontiguousarray(xoT[:, HH:HT]))
        qrows = np.concatenate(
            [xb[128 * (2 * (k - 1) + p):128 * (2 * (k - 1) + p) + 128, :]
             for k in range(1, NSLOT + 1)], axis=0)  # [NQ, E]
        xq_t = _ptile(np.ascontiguousarray(qrows.T).astype(bf))
        in_maps.append({
            "xo0": xo0, "xo1": xo1, "xq": xq_t,
            "m": m_t, "wv": wv_t,
            "maskT": maskT0 if p == 0 else maskT1,
            "ones": ones,
        })
    return in_maps


def _scatter(per_core_out):
    out = np.empty((B, S, D), dtype=np.float32)
    for c in range(NCORES):
        b, p = divmod(c, 2)
        oc = per_core_out[c]                         # [NQ, D]
        for k in range(1, NSLOT + 1):
            g = 2 * (k - 1) + p
            out[b, 128 * g:128 * (g + 1), :] = oc[128 * (k - 1):128 * k, :]
    return out


def kernel(x, w_q, w_k, w_v):
    global _prog
    if _prog is None:
        _prog = _build()
    in_maps = _in_maps(x, w_q, w_k, w_v)
    res = run_bass_kernel_spmd(_prog, in_maps, list(range(NCORES)))
    return _scatter([res.results[c]["out"] for c in range(NCORES)])
